# revision 7
# baseline (speedup 1.0000x reference)
"""Cost-volume kernel for Trainium2 (Bass/Tile), SPMD over 8 NeuronCores.

out[n, c, d, h, x] = l[n, c, h, x] - r[n, c, h, x - d]  for x >= d, else 1.0
shapes: l, r = (2, 32, 128, 256) f32 -> out = (2, 32, 48, 128, 256) f32

Sharding: the 64 (n, c) pairs split 8 ways -> G=8 channels per core; no
cross-device communication.  Output-write bound.

Transfer precision (gate is rel_err < 2e-2): host pre-scales inputs by 8
and casts to fp16 (exact exponent shift; input rounding ~6e-4 rel).  Most
of the volume is written as fp16 (stored value = 8*out, host divides by 8,
exact) - this halves HBM writes vs f32 and doubles DVE throughput (fp16
tensor_tensor runs in 2x mode, ~0.52 ns/elem).  The drain (~405 GB/s) is
still ~16% slower than fp16 production, so the tail ~15% of the volume
(h-half stream 1, disparities 33-47) is written as int8: q = 8*out rounded
to int8 (|8*out| < 70, no saturation; quant err <= 0.0625 abs ~ 7e-3 rel).
int8 dst drops DVE to 1x for those subtracts, which is exactly the slack
needed to balance production against the thinner drain.

Packed valid-region layout: the x < d triangle is a compile-time constant
(1.0) the device never writes.  The volume is split into two h_lo-half
streams packed per partition (g, h_hi) as concat_d [ (4 h_lo rows, x >= d)
block of 4*(W-d) ] .  Splitting by h-half removes the input gate: every
disparity of stream 0 needs only the first half-loads, so DVE has ~23 us
of work queued before the second input halves land (those stream in on the
otherwise-idle GpSimd DGE ring).  DVE subtracts write flat stride-1 dst
ranges (free sizes match; per-dim shapes need not).  Host unpacks + fills
the ones triangle.

DMA shaping (HW-measured on this part): 4 KB descriptors peak (~405
GB/s/core sustained; 2 KB -> 373, 8 KB -> ~190), so DRAM payload rows are
2048 fp16 / 4096 int8 + 16 B pad to pin descriptor size.  Production runs
just above the drain, so the drain head trails production by ~1 row the
whole run: every row goes out as its own 512-KB DMA (measured to sustain
the full 405; coarser chunks stall the rings at chunk boundaries),
alternating the sync/scalar rings.  Partial last rows drain as-is.  The
first 1024 elems go out as their own half-row DMA right after the first
subtract.  ~7 us Tile preamble and ~2.8 us epilogue are fixed.
"""

import numpy as np

import concourse.bacc as bacc
import concourse.mybir as mybir
import concourse.tile as tile
from concourse.bass_utils import run_bass_kernel_spmd

MAX_DISP = 48
N, C, H, W = 2, 32, 128, 256
NCORES = 8
G = (N * C) // NCORES  # 8 (n, c) channels per core
HHI = 16  # partition = (g, h_hi): 8 * 16 = 128
HL = 8  # h_lo rows per partition
HH = HL // 2  # 4 h_lo rows per half-stream

FP = mybir.dt.float16
I8 = mybir.dt.int8
SCALE = 8.0  # power of two: fp16 stores 8*out exactly; int8 q = round(8*out)
D8 = 33  # stream-1 disparities >= D8 are written int8
DSZ = 2048  # fp16 payload elems per DRAM row (4 KB descriptors)
DSZ8 = 4096  # int8 payload elems per DRAM row (4 KB descriptors)
PADW = DSZ + 8
PADW8 = DSZ8 + 16
OFFS = [0]
for _d in range(MAX_DISP):
    OFFS.append(OFFS[-1] + HH * (W - _d))
VH = OFFS[MAX_DISP]  # 44640 elems per half-stream
V16 = VH + OFFS[D8]  # fp16-packed elems: stream0 + stream1 d < D8 (76320)
V8 = VH - OFFS[D8]  # int8-packed elems: stream1 d >= D8 (12960)
R16_0 = -(-VH // DSZ)  # 22 fp16 rows for stream 0 (last: 1632)
R16_1 = -(-OFFS[D8] // DSZ)  # 16 fp16 rows for stream 1 head (last: 960)
NR16 = R16_0 + R16_1  # 38
NR8 = -(-V8 // DSZ8)  # 4 int8 rows (last: 672)

# output DMA plan: (production_watermark_elems, which, row, col0, a, b)
# 'which': 0 = fp16 tensor (src big[a:b]), 1 = int8 tensor (src big8[a:b])
_PLAN = [(1024, 0, 0, 0, 0, 1024), (2048, 0, 0, 1024, 1024, 2048)]
for _r in range(1, R16_0):
    a, b = _r * DSZ, min((_r + 1) * DSZ, VH)
    _PLAN.append((b, 0, _r, 0, a, b))
for _r in range(R16_1):
    a, b = _r * DSZ, min((_r + 1) * DSZ, OFFS[D8])
    _PLAN.append((VH + b, 0, R16_0 + _r, 0, VH + a, VH + b))
for _r in range(NR8):
    a, b = _r * DSZ8, min((_r + 1) * DSZ8, V8)
    _PLAN.append((V16 + b, 1, _r, 0, a, b))

IN_HALF = HH * W  # 1024
IN_PADW = IN_HALF + 4

_CACHE = {}


def build_bass():
    if "nc" in _CACHE:
        return _CACHE["nc"]
    nc = bacc.Bacc("TRN2", target_bir_lowering=False, debug=False)
    l = nc.dram_tensor("l", (G, HHI, 2, IN_PADW), FP, kind="ExternalInput")
    r = nc.dram_tensor("r", (G, HHI, 2, IN_PADW), FP, kind="ExternalInput")
    o16 = nc.dram_tensor("o16", (G, HHI, NR16, PADW), FP, kind="ExternalOutput")
    o8 = nc.dram_tensor("o8", (G, HHI, NR8, PADW8), I8, kind="ExternalOutput")

    with tile.TileContext(nc) as tc:
        with tc.tile_pool(name="sb", bufs=1) as pool:
            l_sb = pool.tile([128, HL, W], FP)
            r_sb = pool.tile([128, HL, W], FP)
            big = pool.tile([128, V16], FP)
            big8 = pool.tile([128, V8], I8)
            # first halves on the output rings (needed in ~3 us); second
            # halves on the GpSimd ring so they never queue ahead of
            # early output rows
            nc.sync.dma_start(out=l_sb[:, :HH], in_=l.ap()[:, :, 0, :IN_HALF])
            nc.scalar.dma_start(out=r_sb[:, :HH], in_=r.ap()[:, :, 0, :IN_HALF])
            nc.gpsimd.dma_start(out=l_sb[:, HH:], in_=l.ap()[:, :, 1, :IN_HALF])
            nc.gpsimd.dma_start(out=r_sb[:, HH:], in_=r.ap()[:, :, 1, :IN_HALF])

            state = {"issue": 0, "sent": 0}

            def flush(ready):
                while state["sent"] < len(_PLAN):
                    wm, which, row, c0, a, b = _PLAN[state["sent"]]
                    if wm > ready:
                        return
                    eng = nc.sync if state["issue"] % 2 == 0 else nc.scalar
                    dst = (o16, o8)[which]
                    src = (big, big8)[which]
                    eng.dma_start(
                        out=dst.ap()[:, :, row, c0 : c0 + (b - a)], in_=src[:, a:b]
                    )
                    state["issue"] += 1
                    state["sent"] += 1

            for s in range(2):
                sl = slice(s * HH, (s + 1) * HH)
                for d in range(MAX_DISP):
                    L = HH * (W - d)
                    if s == 1 and d >= D8:
                        dst = big8[:, OFFS[d] - OFFS[D8] : OFFS[d + 1] - OFFS[D8]]
                    else:
                        base = s * VH
                        dst = big[:, base + OFFS[d] : base + OFFS[d + 1]]
                    nc.vector.tensor_sub(dst, l_sb[:, sl, d:], r_sb[:, sl, : W - d])
                    flush(s * VH + OFFS[d + 1])

    nc.compile()
    _CACHE["nc"] = nc
    return nc


def _pad_rows(x):  # (G, H, W) fp16 -> (G, HHI, 2, IN_PADW)
    flat = x.reshape(G, HHI, 2, IN_HALF)
    padded = np.zeros((G, HHI, 2, IN_PADW), np.float16)
    padded[:, :, :, :IN_HALF] = flat
    return padded


def make_in_maps(l_fmap, r_fmap):
    l_flat = (np.asarray(l_fmap, np.float32) * SCALE).astype(np.float16)
    r_flat = (np.asarray(r_fmap, np.float32) * SCALE).astype(np.float16)
    l_flat = l_flat.reshape(N * C, H, W)
    r_flat = r_flat.reshape(N * C, H, W)
    return [
        {
            "l": _pad_rows(l_flat[k * G : (k + 1) * G]),
            "r": _pad_rows(r_flat[k * G : (k + 1) * G]),
        }
        for k in range(NCORES)
    ]


def gather(results):
    inv = np.float16(1.0 / SCALE)
    out = np.empty((N * C, MAX_DISP, HHI, HL, W), np.float16)
    for k, res in enumerate(results):
        p16 = res["o16"][:, :, :, :DSZ].reshape(G, HHI, NR16 * DSZ)
        p8 = res["o8"][:, :, :, :DSZ8].reshape(G, HHI, NR8 * DSZ8)
        oc = out[k * G : (k + 1) * G]  # (G, D, HHI, HL, W) view
        for s in range(2):
            for d in range(MAX_DISP):
                if s == 1 and d >= D8:
                    a = OFFS[d] - OFFS[D8]
                    seg = p8[:, :, a : a + HH * (W - d)].astype(np.float16)
                else:
                    a = s * R16_0 * DSZ + OFFS[d] if s else OFFS[d]
                    seg = p16[:, :, a : a + HH * (W - d)]
                seg = (seg * inv).reshape(G, HHI, HH, W - d)
                blk = oc[:, d]  # (G, HHI, HL, W) view
                blk[:, :, s * HH : (s + 1) * HH, d:] = seg
                blk[:, :, s * HH : (s + 1) * HH, :d] = np.float16(1.0)
    return out.reshape(N, C, MAX_DISP, H, W).astype(np.float32)


def kernel(l_fmap, r_fmap):
    nc = build_bass()
    in_maps = make_in_maps(l_fmap, r_fmap)
    res = run_bass_kernel_spmd(nc, in_maps, core_ids=list(range(NCORES)))
    return gather(res.results)


# revision 12
# speedup vs baseline: 1.0265x; 1.0265x over previous
"""Cost-volume kernel for Trainium2 (Bass/Tile), SPMD over 8 NeuronCores.

out[n, c, d, h, x] = l[n, c, h, x] - r[n, c, h, x - d]  for x >= d, else 1.0
shapes: l, r = (2, 32, 128, 256) f32 -> out = (2, 32, 48, 128, 256) f32

Sharding: the 64 (n, c) pairs split 8 ways -> G=8 channels per core; no
cross-device communication.  Output-write bound.

Transfer precision (gate is rel_err < 2e-2): host pre-scales inputs by 8
and casts to fp16 (exact exponent shift; input rounding ~6e-4 rel).  Most
of the volume is written as fp16 (stored value = 8*out, host divides by 8,
exact) - this halves HBM writes vs f32 and doubles DVE throughput (fp16
tensor_tensor runs in 2x mode, ~0.52 ns/elem).  The drain (~405 GB/s) is
still ~16% slower than fp16 production, so a small tail (disparities
d >= 44 of both h-half streams, ~7.5% of elems) is written as int8:
q = 8*out rounded to int8 (|8*out| < 70, no saturation; quant err <=
0.0625 abs ~ 7e-3 rel).  int8 dst drops DVE to 1x (~1.25 ns/elem
measured) for those subtracts, so the tail is sized to the point where
the extra DVE time exactly buys back drain time (Pool/Act cannot take
them: Pool has no int8 subtract, and parking casts on the Act queue
blocks its DMA ring).

Packed valid-region layout: the x < d triangle is a compile-time constant
(1.0) the device never writes.  The volume is split into two h_lo-half
streams packed per partition (g, h_hi) as concat_d [ (4 h_lo rows,
x >= d) block of 4*(W-d) ] .  Splitting by h-half removes the input gate:
every disparity of stream 0 needs only the first half-loads, so DVE has
~20 us of work queued before the second input halves land (those stream
in on the GpSimd DGE ring so they never queue ahead of early output
rows).  Subtracts write flat stride-1 dst ranges (free sizes match;
per-dim shapes need not).  Host unpacks + fills the ones triangle.

DMA shaping (HW-measured on this part): 4 KB descriptors peak (~405
GB/s/core sustained; 2 KB -> 373, 8 KB -> ~190), so DRAM payload rows are
2048 fp16 / 4096 int8 + 16 B pad to pin descriptor size.  fp16 production
runs just above the drain, so the drain head trails production by ~1 row
the whole run: every row goes out as its own 512-KB DMA (measured to
sustain the full 405; coarser chunks stall the rings at chunk
boundaries), alternating the sync/scalar rings.  Partial last rows drain
as-is; the int8 rows are emitted last and drain while DVE is already
done.  The first 1024 elems go out as their own half-row DMA right after
the first subtract.  ~7 us Tile preamble and ~2.8 us epilogue are fixed.
"""

import numpy as np

import concourse.bacc as bacc
import concourse.mybir as mybir
import concourse.tile as tile
from concourse.bass_utils import run_bass_kernel_spmd

MAX_DISP = 48
N, C, H, W = 2, 32, 128, 256
NCORES = 8
G = (N * C) // NCORES  # 8 (n, c) channels per core
HHI = 16  # partition = (g, h_hi): 8 * 16 = 128
HL = 8  # h_lo rows per partition
HH = HL // 2  # 4 h_lo rows per half-stream

FP = mybir.dt.float16
I8 = mybir.dt.int8
SCALE = 8.0  # power of two: fp16 stores 8*out exactly; int8 q = round(8*out)
D8 = 44  # disparities >= D8 (both streams) are written int8 (on DVE)
DSZ = 2048  # fp16 payload elems per DRAM row (4 KB descriptors)
DSZ8 = 4096  # int8 payload elems per DRAM row (4 KB descriptors)
PADW = DSZ + 8
PADW8 = DSZ8 + 16
OFFS = [0]
for _d in range(MAX_DISP):
    OFFS.append(OFFS[-1] + HH * (W - _d))
VH = OFFS[MAX_DISP]  # 44640 elems per half-stream
V16S = OFFS[D8]  # fp16 elems per stream (37840)
V8S = VH - V16S  # int8 elems per stream (6800)
R16S = -(-V16S // DSZ)  # 19 fp16 rows per stream (last: 976)
NR16 = 2 * R16S  # 38
NR8 = -(-(2 * V8S) // DSZ8)  # 4 int8 rows (last: 1312)

# fp16 output DMA plan: (production_watermark_elems, row, col0, a, b)
_PLAN = [(1024, 0, 0, 0, 1024), (2048, 0, 1024, 1024, 2048)]
for _r in range(1, R16S):
    a, b = _r * DSZ, min((_r + 1) * DSZ, V16S)
    _PLAN.append((b, _r, 0, a, b))
for _r in range(R16S):
    a, b = _r * DSZ, min((_r + 1) * DSZ, V16S)
    _PLAN.append((V16S + b, R16S + _r, 0, V16S + a, V16S + b))

IN_HALF = HH * W  # 1024
IN_PADW = IN_HALF + 4
IN_PADW8 = IN_HALF + 16

_CACHE = {}


def build_bass():
    if "nc" in _CACHE:
        return _CACHE["nc"]
    nc = bacc.Bacc("TRN2", target_bir_lowering=False, debug=False)
    l = nc.dram_tensor("l", (G, HHI, 2, IN_PADW), FP, kind="ExternalInput")
    r = nc.dram_tensor("r", (G, HHI, 2, IN_PADW), FP, kind="ExternalInput")
    o16 = nc.dram_tensor("o16", (G, HHI, NR16, PADW), FP, kind="ExternalOutput")
    o8 = nc.dram_tensor("o8", (G, HHI, NR8, PADW8), I8, kind="ExternalOutput")

    with tile.TileContext(nc) as tc:
        with tc.tile_pool(name="sb", bufs=1) as pool:
            l_sb = pool.tile([128, HL, W], FP)
            r_sb = pool.tile([128, HL, W], FP)
            big = pool.tile([128, 2 * V16S], FP)
            big8 = pool.tile([128, 2 * V8S], I8)
            # first halves on the output rings (needed in ~3 us); second
            # halves on the GpSimd ring so they never queue ahead of
            # early output rows
            nc.sync.dma_start(out=l_sb[:, :HH], in_=l.ap()[:, :, 0, :IN_HALF])
            nc.scalar.dma_start(out=r_sb[:, :HH], in_=r.ap()[:, :, 0, :IN_HALF])
            nc.gpsimd.dma_start(out=l_sb[:, HH:], in_=l.ap()[:, :, 1, :IN_HALF])
            nc.gpsimd.dma_start(out=r_sb[:, HH:], in_=r.ap()[:, :, 1, :IN_HALF])

            state = {"issue": 0, "sent": 0}

            def dma(dst, src):
                eng = nc.sync if state["issue"] % 2 == 0 else nc.scalar
                eng.dma_start(out=dst, in_=src)
                state["issue"] += 1

            def flush(ready):
                while state["sent"] < len(_PLAN):
                    wm, row, c0, a, b = _PLAN[state["sent"]]
                    if wm > ready:
                        return
                    dma(o16.ap()[:, :, row, c0 : c0 + (b - a)], big[:, a:b])
                    state["sent"] += 1

            for s in range(2):
                sl = slice(s * HH, (s + 1) * HH)
                for d in range(MAX_DISP):
                    if d >= D8:
                        a = s * V8S + OFFS[d] - V16S
                        dst = big8[:, a : a + HH * (W - d)]
                    else:
                        dst = big[:, s * V16S + OFFS[d] : s * V16S + OFFS[d + 1]]
                    nc.vector.tensor_sub(dst, l_sb[:, sl, d:], r_sb[:, sl, : W - d])
                    if d < D8:
                        flush(s * V16S + OFFS[d + 1])

            for _r in range(NR8):
                a, b = _r * DSZ8, min((_r + 1) * DSZ8, 2 * V8S)
                dma(o8.ap()[:, :, _r, : b - a], big8[:, a:b])

    nc.compile()
    _CACHE["nc"] = nc
    return nc


def _pad_rows(x, padw, dt):  # (G, H, W) -> (G, HHI, 2, padw)
    flat = x.reshape(G, HHI, 2, IN_HALF)
    padded = np.zeros((G, HHI, 2, padw), dt)
    padded[:, :, :, :IN_HALF] = flat
    return padded


def make_in_maps(l_fmap, r_fmap):
    l16 = (np.asarray(l_fmap, np.float32) * SCALE).astype(np.float16)
    r16 = (np.asarray(r_fmap, np.float32) * SCALE).astype(np.float16)
    l16 = l16.reshape(N * C, H, W)
    r16 = r16.reshape(N * C, H, W)
    return [
        {
            "l": _pad_rows(l16[k * G : (k + 1) * G], IN_PADW, np.float16),
            "r": _pad_rows(r16[k * G : (k + 1) * G], IN_PADW, np.float16),
        }
        for k in range(NCORES)
    ]


def gather(results):
    inv = np.float16(1.0 / SCALE)
    out = np.empty((N * C, MAX_DISP, HHI, HL, W), np.float16)
    for k, res in enumerate(results):
        p16 = res["o16"][:, :, :, :DSZ].reshape(G, HHI, NR16 * DSZ)
        p8 = res["o8"][:, :, :, :DSZ8].reshape(G, HHI, NR8 * DSZ8)
        oc = out[k * G : (k + 1) * G]  # (G, D, HHI, HL, W) view
        for s in range(2):
            for d in range(MAX_DISP):
                L = HH * (W - d)
                if d >= D8:
                    a = s * V8S + OFFS[d] - V16S
                    seg = p8[:, :, a : a + L].astype(np.float16)
                else:
                    a = s * R16S * DSZ + OFFS[d]
                    seg = p16[:, :, a : a + L]
                seg = (seg * inv).reshape(G, HHI, HH, W - d)
                blk = oc[:, d]  # (G, HHI, HL, W) view
                blk[:, :, s * HH : (s + 1) * HH, d:] = seg
                blk[:, :, s * HH : (s + 1) * HH, :d] = np.float16(1.0)
    return out.reshape(N, C, MAX_DISP, H, W).astype(np.float32)


def kernel(l_fmap, r_fmap):
    nc = build_bass()
    in_maps = make_in_maps(l_fmap, r_fmap)
    res = run_bass_kernel_spmd(nc, in_maps, core_ids=list(range(NCORES)))
    return gather(res.results)
